# revision 25
# baseline (speedup 1.0000x reference)
"""Trainium2 Bass kernel for nn_DLGN_VT (deep linearly-gated network w/ value tensor).

Math (per batch row b):
    g_i = sigmoid(30 * x @ W_i.T)            i = 1,2,3    [B, 32] each
    out[b] = sum_{ijk} g1[b,i] g2[b,j] g3[b,k] V[i,j,k]

Distribution: pure data-parallel over the batch axis, 8 NeuronCores,
512 rows per core. W_i and V are tiny and replicated.

Per-core algorithm (host-side layout prep is free):
  - xc [128, 1216] bf16 = hi/lo bf16 split of the xT shard (x transposed, D on
    partitions) and of Wall^T (order W2;W3;W1).  cba [128, 384] bf16 = V^T
    chunks + S3 selection.  cbb [32, 1025] bf16 = S2 selections + ones.
    Three DMAs, xc first.
  - ~8 dummy matmuls warm the PE HAM clock gate during the DMA wait.
  - Three error-compensated bf16 matmuls (Wh.xh + Wh.xl + Wl.xh) make all 96
    gate logits at fp32-grade accuracy: Gps[96n, 512b].
  - One sigmoid (scale=30) gives g2/g3 bf16 [64, 512]; g1 fp32 later.
  - A^T[(jk), b] = g2[j,b]*g3[k,b] in 4 pair-blocks: two TensorE selection
    matmuls E2 = S2_q.T @ g2t into a 2-bank PSUM tile, then ONE VectorE
    tensor_tensor per pair against E3 = S3.T @ g3t (SBUF).
  - C^T[i, b] = sum_jk V[i,jk] A^T[jk,b] accumulates over 8 bf16 matmuls with
    host-transposed V chunks as the stationary operand.
  - out[b] = sum_i g1t[i,b] * C^T[i,b]: one VectorE multiply + ones-matmul.
"""

import numpy as np
import ml_dtypes

import concourse.bass as bass
import concourse.bacc as bacc
import concourse.mybir as mybir
import concourse.tile as tile
from concourse.alu_op_type import AluOpType
from concourse.bass_utils import run_bass_kernel_spmd

BF16 = ml_dtypes.bfloat16
NCORES = 8
B, D, N = 4096, 128, 32
BL = B // NCORES  # 512 batch rows per core
BETA = 30.0
NQ = 8   # 128-row blocks of the jk=1024 plane
NP = 4   # pairs of blocks

F32 = mybir.dt.float32
DBF = mybir.dt.bfloat16

# xc: packed bf16 input [128, 1216]: xh | xl | Wh | Wl  (bf16x2 split of
# the fp32 xT shard and Wall^T(W2;W3;W1) for an error-compensated bf16 gating
# matmul: logits = Wh.T@xh + Wh.T@xl + Wl.T@xh)
XH0, XH1 = 0, BL
XL0, XL1 = XH1, XH1 + BL
WH0, WH1 = XL1, XL1 + 96
WL0, WL1 = WH1, WH1 + 96
CF1 = WL1
# cba: bf16 [128, 384]: V^T chunks [128, 256] | S3 at rows 32:64, cols 256:384
VT0, VT1 = 0, 256
S30, S31 = 256, 384
# cbb: bf16 [32, 1025]: S2 selections [32, 1024] | ones [32, 1]
S20, S21 = 0, 1024
ON0 = 1024

N_WARMUP = 8  # dummy matmuls to warm the PE HAM clock gate


def build_nc():
    # Bacc (not raw Bass): its compile passes split multi-wait sync infos
    # (TRN2 allows at most one sync wait per compute instruction).
    nc = bacc.Bacc(None)
    xc_d = nc.declare_dram_parameter("xc", [128, CF1], DBF, isOutput=False)
    cba_d = nc.declare_dram_parameter("cba", [128, 384], DBF, isOutput=False)
    cbb_d = nc.declare_dram_parameter("cbb", [32, 1025], DBF, isOutput=False)
    out_d = nc.declare_dram_parameter("out", [1, BL], F32, isOutput=True)

    sig = mybir.ActivationFunctionType.Sigmoid

    with tile.TileContext(nc) as tc:
        with (
            tc.tile_pool(name="const", bufs=1) as cpool,
            tc.tile_pool(name="work", bufs=1) as wpool,
            tc.tile_pool(name="atp", bufs=1) as apool,
            tc.tile_pool(name="psA", bufs=2, space="PSUM") as psA,
            tc.tile_pool(name="psB", bufs=3, space="PSUM") as psB,
        ):
            xc = cpool.tile([128, CF1], DBF)
            cba = cpool.tile([128, 384], DBF)
            cbb = cpool.tile([32, 1025], DBF)
            nc.sync.dma_start(xc[:], xc_d[:])
            nc.sync.dma_start(cba[:], cba_d[:])
            nc.sync.dma_start(cbb[:], cbb_d[:])

            xh = xc[:, XH0:XH1]
            xl = xc[:, XL0:XL1]
            wh = xc[:, WH0:WH1]
            wl = xc[:, WL0:WL1]
            vts = cba[:, VT0:VT1]            # [128, 8*32] V^T chunks (C lhsT)
            s3 = cba[32:64, S30:S31]         # [32, 128] E3 selection (base 32)
            s2 = cbb[:, S20:S21]             # [32, 8*128] E2 selection blocks
            ones = cbb[:, ON0 : ON0 + 1]     # [32, 1]

            # ---- PE warmup in the gating PSUM bank (overwritten later).
            # memset on GpSimd: its queue is free earliest, so the warmup
            # chain starts ~1us sooner than with a DVE memset. ----
            gps = psA.tile([96, BL], F32, tag="ps")
            wz = wpool.tile([128, BL], DBF)
            nc.gpsimd.memset(wz[:], 0.0)
            for _ in range(N_WARMUP):
                nc.tensor.matmul(gps[:, :], wz[:, 0:96], wz[:],
                                 start=True, stop=True)

            # ---- gating: error-compensated bf16 matmul (3 passes at
            # 1 cyc/row vs fp32's 4) -> fp32-grade logits in PSUM ----
            nc.tensor.matmul(gps[:], wh, xh, start=True, stop=False)
            nc.tensor.matmul(gps[:], wh, xl, start=False, stop=False)
            nc.tensor.matmul(gps[:], wl, xh, start=False, stop=True)

            g23 = wpool.tile([2 * N, BL], DBF)
            g1t = wpool.tile([N, BL], F32)
            nc.scalar.activation(g23[:], gps[0:64, :], sig, scale=BETA)
            g2t = g23[0:32, :]
            g3t = g23[32:64, :]  # base partition 32, matching s3

            # ---- E3 = S3.T @ g3t -> SBUF (DVE copy; DVE idle until TTs) ----
            e3ps = psA.tile([128, BL], F32, tag="ps")
            nc.tensor.matmul(e3ps[:], s3, g3t, start=True, stop=True)
            e3s = wpool.tile([128, BL], F32)
            nc.vector.tensor_copy(e3s[:], e3ps[:])

            # ---- A^T blocks: E2 selection matmuls + TTs. Pairs for blocks
            # 0-5 (fewer DVE ops); blocks 6,7 get single TTs so the final C
            # matmuls can start as soon as each half is ready. All emitted
            # before the C matmuls so the PE prioritizes feeding the DVE. ----
            e3b = e3s[:].unsqueeze(1).broadcast_to((128, 2, BL))
            ats = []
            for p in range(NP):
                e2ps = psB.tile([128, 2, BL], F32, tag="e2")  # 2 PSUM banks
                for h in range(2):
                    q = 2 * p + h
                    nc.tensor.matmul(
                        e2ps[:, h, :], s2[:, 128 * q : 128 * (q + 1)], g2t,
                        start=True, stop=True,
                    )
                at = apool.tile([128, 2, BL], DBF, tag=f"at_{p}")
                if p < NP - 1:
                    nc.vector.tensor_tensor(at[:], e2ps[:], e3b, AluOpType.mult)
                else:
                    for h in range(2):
                        nc.vector.tensor_tensor(
                            at[:, h, :], e2ps[:, h, :], e3s[:], AluOpType.mult
                        )
                ats.append(at)

            # ---- g1 sigmoid (off the critical path) ----
            nc.scalar.activation(g1t[:], gps[64:96, :], sig, scale=BETA)

            # ---- C accumulation, split q0-5 / q6-7 so most of the final
            # gated reduce overlaps the TT pipeline (out is linear in C) ----
            cpsA = psA.tile([N, BL], F32, tag="ps")
            cpsB = psA.tile([N, BL], F32, tag="ps")
            for q in range(NQ):
                p, h = q // 2, q % 2
                dst = cpsA if q < 6 else cpsB
                nc.tensor.matmul(
                    dst[:], vts[:, 32 * q : 32 * (q + 1)], ats[p][:, h, :],
                    start=(q in (0, 6)), stop=(q in (5, NQ - 1)),
                )

            # ---- out = ones.T @ (g1t .* C_A) + ones.T @ (g1t .* C_B) ----
            ops = psA.tile([1, BL], F32, tag="ps")
            yA = wpool.tile([N, BL], DBF)
            nc.vector.tensor_tensor(yA[:], cpsA[:], g1t[:], AluOpType.mult)
            nc.tensor.matmul(ops[:], ones, yA[:], start=True, stop=False)
            yB = wpool.tile([N, BL], DBF)
            nc.vector.tensor_tensor(yB[:], cpsB[:], g1t[:], AluOpType.mult)
            nc.tensor.matmul(ops[:], ones, yB[:], start=False, stop=True)
            outs = wpool.tile([1, BL], F32)
            nc.scalar.copy(outs[:], ops[:])
            nc.sync.dma_start(out_d[:], outs[:])

    nc.finalize()
    return nc


def host_prep(x, W1, W2, W3, V):
    """Build per-core input maps (all numpy, fp32 in / packed layouts out)."""
    x = np.asarray(x, dtype=np.float32)
    W1 = np.asarray(W1, dtype=np.float32)
    W2 = np.asarray(W2, dtype=np.float32)
    W3 = np.asarray(W3, dtype=np.float32)
    V = np.asarray(V, dtype=np.float32)

    xT = np.ascontiguousarray(x.T)  # [128, 4096]

    # order: g2 rows first (E2-mm rhs at base partition 0), then g3 (base 32,
    # matching the S3 placement), then g1 (only needed at the very end)
    Wall = np.concatenate([W2, W3, W1], axis=0)  # [96, 128]
    cf = np.ascontiguousarray(Wall.T)  # [128, 96] fp32

    # V^T chunks: VTs[p, 32q + i] = V[0, i, j, k] with jk = 128q + p
    Vr = V.reshape(N, N * N)  # [i, jk]
    VT = np.ascontiguousarray(Vr.T)  # [jk, i]
    VTs = VT.reshape(NQ, 128, N).transpose(1, 0, 2).reshape(128, NQ * N)

    # E2 selection: S2[j', q*128 + p] = 1 iff j' == 4q + p//32
    S2 = np.zeros((N, NQ, 128), dtype=np.float32)
    for q in range(NQ):
        for p in range(128):
            S2[4 * q + p // 32, q, p] = 1.0
    S2pack = S2.reshape(N, NQ * 128)

    # E3 selection: S3[k', p] = 1 iff k' == p % 32
    S3 = np.zeros((N, 128), dtype=np.float32)
    for p in range(128):
        S3[p % 32, p] = 1.0

    cba = np.zeros((128, 384), dtype=BF16)
    cba[:, VT0:VT1] = VTs.astype(BF16)
    cba[32:64, S30:S31] = S3.astype(BF16)

    cbb = np.zeros((32, 1025), dtype=BF16)
    cbb[:, S20:S21] = S2pack.astype(BF16)
    cbb[:, ON0] = np.ones(N, dtype=BF16)

    wh = cf.astype(BF16)
    wl = (cf - wh.astype(np.float32)).astype(BF16)

    xc = np.zeros((128, CF1), dtype=BF16)
    xc[:, WH0:WH1] = wh
    xc[:, WL0:WL1] = wl

    in_maps = []
    for c in range(NCORES):
        m = xc.copy()
        xs = xT[:, c * BL : (c + 1) * BL]
        xhc = xs.astype(BF16)
        m[:, XH0:XH1] = xhc
        m[:, XL0:XL1] = (xs - xhc.astype(np.float32)).astype(BF16)
        in_maps.append({"xc": m, "cba": cba, "cbb": cbb})
    return in_maps


_CACHED_NC = None


def _ensure_ntff_hook():
    """The agent image's `antenv` package lacks `axon_hooks`; synthesize it
    and register the boot module's ctypes-based NTFF profile hook so
    run_bass_kernel_spmd(trace=True) can capture neuron-profile output."""
    import sys, types

    try:
        from antenv.axon_hooks import get_axon_ntff_profile_hook  # noqa: F401

        return
    except ImportError:
        pass
    import antenv
    from trn_agent_boot.trn_boot import _ntff_profile_via_ctypes

    mod = types.ModuleType("antenv.axon_hooks")
    mod._hook = _ntff_profile_via_ctypes("/opt/axon/libaxon_pjrt.so")
    mod.get_axon_ntff_profile_hook = lambda: mod._hook
    mod.set_axon_ntff_profile_hook = lambda h: setattr(mod, "_hook", h)
    sys.modules["antenv.axon_hooks"] = mod
    antenv.axon_hooks = mod


def run(inputs, trace=False, **trace_kwargs):
    """Run the kernel on 8 cores. Returns (out [4096] f32, BassKernelResults)."""
    global _CACHED_NC
    if trace:
        _ensure_ntff_hook()
    if _CACHED_NC is None:
        _CACHED_NC = build_nc()
    in_maps = host_prep(
        inputs["x"], inputs["W1"], inputs["W2"], inputs["W3"], inputs["V"]
    )
    res = run_bass_kernel_spmd(
        _CACHED_NC, in_maps, core_ids=list(range(NCORES)), trace=trace, **trace_kwargs
    )
    out = np.concatenate(
        [np.asarray(res.results[c]["out"]).reshape(BL) for c in range(NCORES)]
    ).astype(np.float32)
    return out, res


def kernel(**inputs):
    out, _ = run(inputs, trace=False)
    return out


# revision 29
# speedup vs baseline: 1.0392x; 1.0392x over previous
"""Trainium2 Bass kernel for nn_DLGN_VT (deep linearly-gated network w/ value tensor).

Math (per batch row b):
    g_i = sigmoid(30 * x @ W_i.T)            i = 1,2,3    [B, 32] each
    out[b] = sum_{ijk} g1[b,i] g2[b,j] g3[b,k] V[i,j,k]

Distribution: pure data-parallel over the batch axis, 8 NeuronCores,
512 rows per core. W_i and V are tiny and replicated.

Per-core algorithm (host-side layout prep is free):
  - xc [128, 1216] bf16 = hi/lo bf16 split of the xT shard (x transposed, D on
    partitions) and of Wall^T (order W2;W3;W1).  cba [128, 384] bf16 = V^T
    chunks + S3 selection.  cbb [32, 1025] bf16 = S2 selections + ones.
    Three DMAs, xc first.
  - ~8 dummy matmuls warm the PE HAM clock gate during the DMA wait.
  - Three error-compensated bf16 matmuls (Wh.xh + Wh.xl + Wl.xh) make all 96
    gate logits at fp32-grade accuracy: Gps[96n, 512b].
  - One sigmoid (scale=30) gives g2/g3 bf16 [64, 512]; g1 fp32 later.
  - A^T[(jk), b] = g2[j,b]*g3[k,b] in 4 pair-blocks: two TensorE selection
    matmuls E2 = S2_q.T @ g2t into a 2-bank PSUM tile, then ONE VectorE
    tensor_tensor per pair against E3 = S3.T @ g3t (SBUF).
  - C^T[i, b] = sum_jk V[i,jk] A^T[jk,b] accumulates over 8 bf16 matmuls with
    host-transposed V chunks as the stationary operand.
  - out[b] = sum_i g1t[i,b] * C^T[i,b]: one VectorE multiply + ones-matmul.
"""

import numpy as np
import ml_dtypes

import concourse.bass as bass
import concourse.bacc as bacc
import concourse.mybir as mybir
import concourse.tile as tile
from concourse.alu_op_type import AluOpType
from concourse.bass_utils import run_bass_kernel_spmd

BF16 = ml_dtypes.bfloat16
NCORES = 8
B, D, N = 4096, 128, 32
BL = B // NCORES  # 512 batch rows per core
BETA = 30.0
NQ = 8   # 128-row blocks of the jk=1024 plane
NP = 4   # pairs of blocks

F32 = mybir.dt.float32
DBF = mybir.dt.bfloat16

# xc: packed bf16 input [128, 1216]: xh | xl | Wh | Wl  (bf16x2 split of
# the fp32 xT shard and Wall^T(W2;W3;W1) for an error-compensated bf16 gating
# matmul: logits = Wh.T@xh + Wh.T@xl + Wl.T@xh)
XH0, XH1 = 0, BL
XL0, XL1 = XH1, XH1 + BL
WH0, WH1 = XL1, XL1 + 96
WL0, WL1 = WH1, WH1 + 96
CF1 = WL1
# cba: bf16 [128, 384]: V^T chunks [128, 256] | S3 at rows 32:64, cols 256:384
VT0, VT1 = 0, 256
S30, S31 = 256, 384
# cbb: bf16 [32, 1025]: S2 selections [32, 1024] | ones [32, 1]
S20, S21 = 0, 1024
ON0 = 1024

N_WARMUP = 7  # dummy matmuls to warm the PE HAM clock gate


def build_nc():
    # Bacc (not raw Bass): its compile passes split multi-wait sync infos
    # (TRN2 allows at most one sync wait per compute instruction).
    nc = bacc.Bacc(None)
    xc_d = nc.declare_dram_parameter("xc", [128, CF1], DBF, isOutput=False)
    cba_d = nc.declare_dram_parameter("cba", [128, 384], DBF, isOutput=False)
    cbb_d = nc.declare_dram_parameter("cbb", [32, 1025], DBF, isOutput=False)
    out_d = nc.declare_dram_parameter("out", [1, BL], F32, isOutput=True)

    sig = mybir.ActivationFunctionType.Sigmoid

    with tile.TileContext(nc) as tc:
        with (
            tc.tile_pool(name="const", bufs=1) as cpool,
            tc.tile_pool(name="work", bufs=1) as wpool,
            tc.tile_pool(name="atp", bufs=1) as apool,
            tc.tile_pool(name="psA", bufs=2, space="PSUM") as psA,
            tc.tile_pool(name="psB", bufs=3, space="PSUM") as psB,
        ):
            xc = cpool.tile([128, CF1], DBF)
            cba = cpool.tile([128, 384], DBF)
            cbb = cpool.tile([32, 1025], DBF)
            nc.sync.dma_start(xc[:], xc_d[:])
            nc.sync.dma_start(cba[:], cba_d[:])
            nc.sync.dma_start(cbb[:], cbb_d[:])

            xh = xc[:, XH0:XH1]
            xl = xc[:, XL0:XL1]
            wh = xc[:, WH0:WH1]
            wl = xc[:, WL0:WL1]
            vts = cba[:, VT0:VT1]            # [128, 8*32] V^T chunks (C lhsT)
            s3 = cba[32:64, S30:S31]         # [32, 128] E3 selection (base 32)
            s2 = cbb[:, S20:S21]             # [32, 8*128] E2 selection blocks
            ones = cbb[:, ON0 : ON0 + 1]     # [32, 1]

            # ---- PE warmup in the gating PSUM bank (overwritten later).
            # memset on GpSimd: its queue is free earliest, so the warmup
            # chain starts ~1us sooner than with a DVE memset. ----
            gps = psA.tile([96, BL], F32, tag="ps")
            wz = wpool.tile([128, BL], DBF)
            nc.gpsimd.memset(wz[:], 0.0)
            for _ in range(N_WARMUP):
                nc.tensor.matmul(gps[:, :], wz[:, 0:96], wz[:],
                                 start=True, stop=True)

            # ---- gating: error-compensated bf16 matmul (3 passes at
            # 1 cyc/row vs fp32's 4) -> fp32-grade logits in PSUM ----
            nc.tensor.matmul(gps[:], wh, xh, start=True, stop=False)
            nc.tensor.matmul(gps[:], wh, xl, start=False, stop=False)
            nc.tensor.matmul(gps[:], wl, xh, start=False, stop=True)

            g23 = wpool.tile([2 * N, BL], DBF)
            g1t = wpool.tile([N, BL], F32)
            nc.scalar.activation(g23[:], gps[0:64, :], sig, scale=BETA)
            g2t = g23[0:32, :]
            g3t = g23[32:64, :]  # base partition 32, matching s3

            # ---- E3 = S3.T @ g3t -> SBUF. The PSUM->SBUF copy is the gate
            # for the first TT, so split it across the idle DVE and ACT ----
            e3ps = psA.tile([128, BL], F32, tag="ps")
            nc.tensor.matmul(e3ps[:], s3, g3t, start=True, stop=True)
            e3s = wpool.tile([128, BL], F32)
            HB = BL // 2
            nc.vector.tensor_copy(e3s[:, 0:HB], e3ps[:, 0:HB])
            nc.scalar.copy(e3s[:, HB:BL], e3ps[:, HB:BL])

            # ---- A^T pair-blocks: E2 selection matmuls + one TT per pair.
            # All pairs emitted before the C matmuls so the PE prioritizes
            # feeding the DVE (TTs are the pipeline bottleneck). ----
            ats = []
            for p in range(NP):
                e2ps = psB.tile([128, 2, BL], F32, tag="e2")  # 2 PSUM banks
                for h in range(2):
                    q = 2 * p + h
                    nc.tensor.matmul(
                        e2ps[:, h, :], s2[:, 128 * q : 128 * (q + 1)], g2t,
                        start=True, stop=True,
                    )
                at = apool.tile([128, 2, BL], DBF, tag=f"at_{p}")
                e3b = e3s[:].unsqueeze(1).broadcast_to((128, 2, BL))
                nc.vector.tensor_tensor(at[:], e2ps[:], e3b, AluOpType.mult)
                ats.append(at)

            # ---- g1 sigmoid (off the critical path) ----
            nc.scalar.activation(g1t[:], gps[64:96, :], sig, scale=BETA)

            # ---- C accumulation over the 8 blocks ----
            cps = psA.tile([N, BL], F32, tag="ps")
            for q in range(NQ):
                p, h = q // 2, q % 2
                nc.tensor.matmul(
                    cps[:], vts[:, 32 * q : 32 * (q + 1)], ats[p][:, h, :],
                    start=(q == 0), stop=(q == NQ - 1),
                )

            # ---- out = ones.T @ (g1t .* C^T); final PSUM->SBUF copy is also
            # split ACT/DVE (both idle) to shorten the tail ----
            y = wpool.tile([N, BL], DBF)
            nc.vector.tensor_tensor(y[:], cps[:], g1t[:], AluOpType.mult)
            ops = psA.tile([1, BL], F32, tag="ps")
            nc.tensor.matmul(ops[:], ones, y[:], start=True, stop=True)
            outs = wpool.tile([1, BL], F32)
            nc.scalar.copy(outs[:, 0:HB], ops[:, 0:HB])
            nc.vector.tensor_copy(outs[:, HB:BL], ops[:, HB:BL])
            nc.sync.dma_start(out_d[:], outs[:])

    nc.finalize()
    return nc


def host_prep(x, W1, W2, W3, V):
    """Build per-core input maps (all numpy, fp32 in / packed layouts out)."""
    x = np.asarray(x, dtype=np.float32)
    W1 = np.asarray(W1, dtype=np.float32)
    W2 = np.asarray(W2, dtype=np.float32)
    W3 = np.asarray(W3, dtype=np.float32)
    V = np.asarray(V, dtype=np.float32)

    xT = np.ascontiguousarray(x.T)  # [128, 4096]

    # order: g2 rows first (E2-mm rhs at base partition 0), then g3 (base 32,
    # matching the S3 placement), then g1 (only needed at the very end)
    Wall = np.concatenate([W2, W3, W1], axis=0)  # [96, 128]
    cf = np.ascontiguousarray(Wall.T)  # [128, 96] fp32

    # V^T chunks: VTs[p, 32q + i] = V[0, i, j, k] with jk = 128q + p
    Vr = V.reshape(N, N * N)  # [i, jk]
    VT = np.ascontiguousarray(Vr.T)  # [jk, i]
    VTs = VT.reshape(NQ, 128, N).transpose(1, 0, 2).reshape(128, NQ * N)

    # E2 selection: S2[j', q*128 + p] = 1 iff j' == 4q + p//32
    S2 = np.zeros((N, NQ, 128), dtype=np.float32)
    for q in range(NQ):
        for p in range(128):
            S2[4 * q + p // 32, q, p] = 1.0
    S2pack = S2.reshape(N, NQ * 128)

    # E3 selection: S3[k', p] = 1 iff k' == p % 32
    S3 = np.zeros((N, 128), dtype=np.float32)
    for p in range(128):
        S3[p % 32, p] = 1.0

    cba = np.zeros((128, 384), dtype=BF16)
    cba[:, VT0:VT1] = VTs.astype(BF16)
    cba[32:64, S30:S31] = S3.astype(BF16)

    cbb = np.zeros((32, 1025), dtype=BF16)
    cbb[:, S20:S21] = S2pack.astype(BF16)
    cbb[:, ON0] = np.ones(N, dtype=BF16)

    wh = cf.astype(BF16)
    wl = (cf - wh.astype(np.float32)).astype(BF16)

    xc = np.zeros((128, CF1), dtype=BF16)
    xc[:, WH0:WH1] = wh
    xc[:, WL0:WL1] = wl

    in_maps = []
    for c in range(NCORES):
        m = xc.copy()
        xs = xT[:, c * BL : (c + 1) * BL]
        xhc = xs.astype(BF16)
        m[:, XH0:XH1] = xhc
        m[:, XL0:XL1] = (xs - xhc.astype(np.float32)).astype(BF16)
        in_maps.append({"xc": m, "cba": cba, "cbb": cbb})
    return in_maps


_CACHED_NC = None


def _ensure_ntff_hook():
    """The agent image's `antenv` package lacks `axon_hooks`; synthesize it
    and register the boot module's ctypes-based NTFF profile hook so
    run_bass_kernel_spmd(trace=True) can capture neuron-profile output."""
    import sys, types

    try:
        from antenv.axon_hooks import get_axon_ntff_profile_hook  # noqa: F401

        return
    except ImportError:
        pass
    import antenv
    from trn_agent_boot.trn_boot import _ntff_profile_via_ctypes

    mod = types.ModuleType("antenv.axon_hooks")
    mod._hook = _ntff_profile_via_ctypes("/opt/axon/libaxon_pjrt.so")
    mod.get_axon_ntff_profile_hook = lambda: mod._hook
    mod.set_axon_ntff_profile_hook = lambda h: setattr(mod, "_hook", h)
    sys.modules["antenv.axon_hooks"] = mod
    antenv.axon_hooks = mod


def run(inputs, trace=False, **trace_kwargs):
    """Run the kernel on 8 cores. Returns (out [4096] f32, BassKernelResults)."""
    global _CACHED_NC
    if trace:
        _ensure_ntff_hook()
    if _CACHED_NC is None:
        _CACHED_NC = build_nc()
    in_maps = host_prep(
        inputs["x"], inputs["W1"], inputs["W2"], inputs["W3"], inputs["V"]
    )
    res = run_bass_kernel_spmd(
        _CACHED_NC, in_maps, core_ids=list(range(NCORES)), trace=trace, **trace_kwargs
    )
    out = np.concatenate(
        [np.asarray(res.results[c]["out"]).reshape(BL) for c in range(NCORES)]
    ).astype(np.float32)
    return out, res


def kernel(**inputs):
    out, _ = run(inputs, trace=False)
    return out
